# revision 1
# baseline (speedup 1.0000x reference)
"""Additive attention (Bahdanau) Trainium2 kernel, 8-core data parallel.

out = softmax_k(mask(sum_h w_v[h] * tanh(q@Wq [q,h] + k@Wk [k,h]))) @ V

Per-core work (2 batches): dominated by tanh over 2*64*512*256 = 16.8M
elements on the Scalar (ACT) engine -> ~110us floor.  Pipeline:
  DVE:  sum slab  s[h, q, k] = kfT[h,k] + qfT[h,q]   (bf16 tensor_scalar)
  ACT:  tanh over big slabs (several q's per instruction)
  PE :  score rows via accumulating one-hot matmuls
        lhsT_q = w_v (x) e_q  ->  psum[q, :] += w_v . tanh_feat_q
  then masked exp (bias rows from valid_lens fused into the psum
  evacuation, softmax denominator from the exp's accum_out), row
  normalization, PE-transpose of attn, attn.T stationary @ V.

Engines execute their streams in order, so emission order is tuned so the
first tanh slab issues early (critical DMAs first on two queues) and
batch 1's transposes/projections are emitted before batch 0's softmax.
"""

import os
from contextlib import ExitStack

import ml_dtypes
import numpy as np

import concourse.bacc as bacc
import concourse.bass as bass
import concourse.mybir as mybir
import concourse.tile as tile
from concourse.bass_utils import run_bass_kernel_spmd

F32 = mybir.dt.float32
BF16 = mybir.dt.bfloat16
I32 = mybir.dt.int32
AF = mybir.ActivationFunctionType
ALU = mybir.AluOpType

B, NQ, NK, QS, KS, H, VD = 16, 64, 512, 256, 256, 256, 256
NCORES = 8
BPC = B // NCORES  # batches per core
MASK_NEG = -30.0  # exp(-30+5) ~ 1e-11 of any valid term; scores are in [-5, 5]

CHUNKS_B0 = [2, 2, 4, 8] + [12] * 4
CHUNKS_B1 = [12] * 4 + [8, 4, 4]


def _build():
    nc = bacc.Bacc()
    q_d = nc.declare_dram_parameter("queries", [BPC, NQ, QS], F32, isOutput=False)
    k_d = nc.declare_dram_parameter("keys", [BPC, NK, KS], F32, isOutput=False)
    v_d = nc.declare_dram_parameter("values", [BPC, NK, VD], F32, isOutput=False)
    vl_d = nc.declare_dram_parameter("valid_lens", [BPC, 1], I32, isOutput=False)
    wq_d = nc.declare_dram_parameter("W_q", [QS, H], F32, isOutput=False)
    wk_d = nc.declare_dram_parameter("W_k", [KS, H], F32, isOutput=False)
    wv_d = nc.declare_dram_parameter("w_v", [H], F32, isOutput=False)
    out_d = nc.declare_dram_parameter("out", [BPC, NQ, VD], F32, isOutput=True)

    # compile-time constants baked into the NEFF
    ident_d = nc.inline_tensor(np.eye(128, dtype=np.float32), name="ident_c")
    identb_d = nc.inline_tensor(
        np.eye(128).astype(ml_dtypes.bfloat16), name="identb_c"
    )
    # one-hot pattern for the score-reduction weights
    diag_d = nc.inline_tensor(
        np.eye(NQ, NQ).astype(ml_dtypes.bfloat16).reshape(NQ * NQ), name="diag_c"
    )
    krow_d = nc.inline_tensor(np.arange(NK, dtype=np.float32), name="krow_c")

    with ExitStack() as ctx:
        tc = ctx.enter_context(tile.TileContext(nc))
        consts = ctx.enter_context(tc.tile_pool(name="consts", bufs=1))
        setup = ctx.enter_context(tc.tile_pool(name="setup", bufs=2))
        slabs = ctx.enter_context(tc.tile_pool(name="slabs", bufs=2))
        sm = ctx.enter_context(tc.tile_pool(name="sm", bufs=1))
        outp = ctx.enter_context(tc.tile_pool(name="outp", bufs=2))
        ps_sc = ctx.enter_context(tc.tile_pool(name="ps_sc", bufs=2, space="PSUM"))
        ps_misc = ctx.enter_context(tc.tile_pool(name="ps_misc", bufs=2, space="PSUM"))
        ps_out = ctx.enter_context(tc.tile_pool(name="ps_out", bufs=2, space="PSUM"))

        # ---------------- loads (critical first, two queues) ----------------
        # batch-0 key blocks split across both queues so they land in parallel
        k_sb0 = setup.tile([128, 4, KS], F32, tag="k_sb0", bufs=1)
        k0_view = k_d[0].rearrange("(kb p) d -> p kb d", p=128)
        ident = consts.tile([128, 128], F32)
        # half-block granularity, alternating queues, so the first transpose
        # can start as early as possible and the rest stream in behind it
        nc.sync.dma_start(out=k_sb0[:, 0, 0:128], in_=k0_view[:, 0, 0:128])
        nc.gpsimd.dma_start(out=ident, in_=ident_d[:, :])
        nc.gpsimd.dma_start(out=k_sb0[:, 0, 128:256], in_=k0_view[:, 0, 128:256])
        nc.sync.dma_start(out=k_sb0[:, 1, 0:128], in_=k0_view[:, 1, 0:128])
        nc.gpsimd.dma_start(out=k_sb0[:, 1, 128:256], in_=k0_view[:, 1, 128:256])
        nc.sync.dma_start(out=k_sb0[:, 2, 0:128], in_=k0_view[:, 2, 0:128])
        nc.gpsimd.dma_start(out=k_sb0[:, 2, 128:256], in_=k0_view[:, 2, 128:256])
        nc.sync.dma_start(out=k_sb0[:, 3, 0:128], in_=k0_view[:, 3, 0:128])
        nc.gpsimd.dma_start(out=k_sb0[:, 3, 128:256], in_=k0_view[:, 3, 128:256])
        q_sb0 = setup.tile([NQ, QS], F32, tag="q_sb0", bufs=1)
        nc.sync.dma_start(out=q_sb0, in_=q_d[0])
        wq_sb = setup.tile([128, 2, H], F32, tag="wq_f", bufs=1)
        nc.sync.dma_start(out=wq_sb, in_=wq_d.rearrange("(kt p) m -> p kt m", p=128))
        wk_sb = setup.tile([128, 2, H], F32, tag="wk_f", bufs=1)
        nc.gpsimd.dma_start(out=wk_sb, in_=wk_d.rearrange("(kt p) m -> p kt m", p=128))
        wv_col = consts.tile([128, 2], F32)
        nc.gpsimd.dma_start(out=wv_col, in_=wv_d.rearrange("(t p) -> p t", p=128))
        diag_bf = consts.tile([128, NQ, NQ], BF16)
        nc.sync.dma_start(out=diag_bf, in_=diag_d[None, :].partition_broadcast(128))
        identb = consts.tile([128, 128], BF16)
        nc.gpsimd.dma_start(out=identb, in_=identb_d[:, :])
        k_sb1 = setup.tile([128, 4, KS], F32, tag="k_sb1", bufs=1)
        for kb in range(4):
            nc.sync.dma_start(
                out=k_sb1[:, kb],
                in_=k_d[1].rearrange("(kb p) d -> p kb d", p=128)[:, kb],
            )
        q_sb1 = setup.tile([NQ, QS], F32, tag="q_sb1", bufs=1)
        nc.sync.dma_start(out=q_sb1, in_=q_d[1])
        krow = consts.tile([128, NK], F32)
        nc.sync.dma_start(out=krow, in_=krow_d[None, :].partition_broadcast(128))
        v_sbs, valid_sbs = [], []
        for b in range(BPC):
            v_sb = setup.tile([128, 4, VD], F32, tag=f"v_sb{b}", name=f"v_sb{b}", bufs=1)
            nc.gpsimd.dma_start(
                out=v_sb, in_=v_d[b].rearrange("(kb p) d -> p kb d", p=128)
            )
            v_sbs.append(v_sb)
            valid_sb = setup.tile([128, 1], I32, tag=f"valid{b}", name=f"valid{b}")
            nc.gpsimd.dma_start(
                out=valid_sb, in_=vl_d[b : b + 1, :].partition_broadcast(128)
            )
            valid_sbs.append(valid_sb)

        k_sbs = [k_sb0, k_sb1]
        q_sbs = [q_sb0, q_sb1]

        # projection weights to bf16 (first on the DVE stream; their DMAs
        # are early on the gpsimd queue)
        wq_bf = consts.tile([128, 2, H], BF16)
        wk_bf = consts.tile([128, 2, H], BF16)
        for kt in range(2):
            nc.vector.tensor_copy(out=wk_bf[:, kt], in_=wk_sb[:, kt])
            nc.vector.tensor_copy(out=wq_bf[:, kt], in_=wq_sb[:, kt])

        onehot = consts.tile([128, 2, NQ, NQ], BF16)

        def setup_batch(b):
            """transposes + projections for batch b -> (kfT_bf, qfT_f32)"""
            k_sb, q_sb = k_sbs[b], q_sbs[b]
            kT_bf = setup.tile([128, 2, NK], BF16, tag="kT", name=f"kT{b}")
            for kb in range(4):
                for kt in range(2):
                    pst = ps_misc.tile(
                        [128, 512], F32, tag="ps_misc", name="pst_k"
                    )
                    nc.tensor.transpose(
                        pst[:, 0:128], k_sb[:, kb, kt * 128 : (kt + 1) * 128], ident
                    )
                    nc.vector.tensor_copy(
                        out=kT_bf[:, kt, kb * 128 : (kb + 1) * 128], in_=pst[:, 0:128]
                    )
            qT_bf = setup.tile([128, 2, NQ], BF16, tag="qT", name=f"qT{b}")
            for kt in range(2):
                pst = ps_misc.tile([128, 512], F32, tag="ps_misc", name="pst_q")
                nc.tensor.transpose(
                    pst[:, 0:NQ], q_sb[:, kt * 128 : (kt + 1) * 128], ident[0:NQ, 0:NQ]
                )
                nc.vector.tensor_copy(out=qT_bf[:, kt, :], in_=pst[:, 0:NQ])

            kfT_bf = setup.tile([128, 2, NK], BF16, tag="kfT", name=f"kfT{b}")
            for mt in range(2):
                psp = ps_misc.tile([128, 512], F32, tag="ps_misc", name="psp_k")
                for kt in range(2):
                    nc.tensor.matmul(
                        psp,
                        lhsT=wk_bf[:, kt, mt * 128 : (mt + 1) * 128],
                        rhs=kT_bf[:, kt, :],
                        start=(kt == 0),
                        stop=(kt == 1),
                    )
                if b == 0:
                    # ACT is idle before the first tanh slab: evacuate there
                    # to shorten the DVE critical path into the first adds
                    nc.scalar.copy(out=kfT_bf[:, mt], in_=psp)
                else:
                    nc.vector.tensor_copy(out=kfT_bf[:, mt], in_=psp)
            qfT_f32 = setup.tile([128, 2, NQ], F32, tag="qfTf", name=f"qfT{b}")
            for mt in range(2):
                psp = ps_misc.tile([128, 512], F32, tag="ps_misc", name="psp_q")
                for kt in range(2):
                    nc.tensor.matmul(
                        psp[:, 0:NQ],
                        lhsT=wq_bf[:, kt, mt * 128 : (mt + 1) * 128],
                        rhs=qT_bf[:, kt, :],
                        start=(kt == 0),
                        stop=(kt == 1),
                    )
                nc.vector.tensor_copy(out=qfT_f32[:, mt], in_=psp[:, 0:NQ])
            return kfT_bf, qfT_f32

        def feature_loop(b, kfT_bf, qfT_f32, sc_ps, build_onehot):
            chunks = CHUNKS_B0 if b == 0 else CHUNKS_B1
            first = [True]
            q0 = 0
            for ci, qn in enumerate(chunks):
                feat = slabs.tile([128, 12, 2, NK], BF16, tag="feat", name="feat")
                sum_bf = slabs.tile([128, 12, 2, NK], BF16, tag="sum", name="sum")
                for qi in range(qn):
                    q = q0 + qi
                    for ht in range(2):
                        nc.vector.tensor_scalar_add(
                            out=sum_bf[:, qi, ht],
                            in0=kfT_bf[:, ht],
                            scalar1=qfT_f32[:, ht, q : q + 1],
                        )
                nc.scalar.activation(
                    out=feat[:, 0:qn], in_=sum_bf[:, 0:qn], func=AF.Tanh
                )
                if build_onehot and ci == 0:
                    for ht in range(2):
                        nc.vector.tensor_scalar_mul(
                            out=onehot[:, ht],
                            in0=diag_bf,
                            scalar1=wv_col[:, ht : ht + 1],
                        )
                for qi in range(qn):
                    q = q0 + qi
                    for ht in range(2):
                        nc.tensor.matmul(
                            sc_ps[0:NQ],
                            lhsT=onehot[:, ht, q],
                            rhs=feat[:, qi, ht],
                            start=first[0],
                            stop=(ci == len(chunks) - 1 and qi == qn - 1 and ht == 1),
                        )
                        first[0] = False
                q0 += qn

        def finish_batch(b, sc_ps):
            # all tensors stay 128-row (rows 32g+o, o<16 are real queries,
            # the rest benign zero-score rows); free-dim-paced engine cost
            # is identical and the layout stays partition-aligned
            valid_f = setup.tile([128, 1], F32, tag="validf", name=f"vf{b}")
            nc.vector.tensor_copy(out=valid_f, in_=valid_sbs[b])
            bias_b = setup.tile([128, NK], F32, tag="bias", name=f"bias{b}")
            nc.vector.tensor_scalar(
                out=bias_b, in0=krow, scalar1=valid_f[:, 0:1], scalar2=None,
                op0=ALU.is_lt,
            )
            nc.vector.tensor_scalar(
                out=bias_b, in0=bias_b, scalar1=1.0, scalar2=-MASK_NEG,
                op0=ALU.subtract, op1=ALU.mult,
            )
            sc_sb = sm.tile([NQ, NK], F32, tag=f"scsb{b}", name=f"scsb{b}")
            nc.vector.tensor_tensor(
                out=sc_sb, in0=sc_ps[0:NQ], in1=bias_b[0:NQ], op=ALU.add
            )
            e_sb = sm.tile([NQ, NK], F32, tag=f"e{b}", name=f"e{b}")
            denom = sm.tile([NQ, 1], F32, tag=f"den{b}", name=f"den{b}")
            nc.scalar.activation(out=e_sb, in_=sc_sb, func=AF.Exp, accum_out=denom)
            recip = sm.tile([NQ, 1], F32, tag=f"rec{b}", name=f"rec{b}")
            nc.vector.reciprocal(recip, denom)
            attn = sm.tile([NQ, NK], BF16, tag=f"at{b}", name=f"at{b}")
            nc.vector.tensor_scalar_mul(out=attn, in0=e_sb, scalar1=recip[:, 0:1])
            v_bf = outp.tile([128, 4, VD], BF16, tag="v_bf", name=f"v_bf{b}")
            for kb in range(4):
                nc.vector.tensor_copy(out=v_bf[:, kb], in_=v_sbs[b][:, kb])

            attnT = outp.tile([128, 4, NQ], BF16, tag="attnT", name=f"attnT{b}")
            for kb in range(4):
                pst = ps_misc.tile(
                    [128, 1024], BF16, tag="ps_misc_b", name="pst_a"
                )
                nc.tensor.transpose(
                    pst[:, 0:NQ],
                    attn[:, kb * 128 : (kb + 1) * 128],
                    identb[0:NQ, 0:NQ],
                )
                nc.vector.tensor_copy(out=attnT[:, kb], in_=pst[:, 0:NQ])

            po = ps_out.tile([NQ, VD], F32, tag="po", name=f"po{b}")
            for kb in range(4):
                nc.tensor.matmul(
                    po,
                    lhsT=attnT[:, kb],
                    rhs=v_bf[:, kb],
                    start=(kb == 0),
                    stop=(kb == 3),
                )
            o_sb = outp.tile([NQ, VD], F32, tag="o_sb", name=f"o_sb{b}")
            nc.vector.tensor_copy(out=o_sb, in_=po)
            nc.gpsimd.dma_start(out=out_d[b], in_=o_sb)

        interleave = os.environ.get("ATTN_INTERLEAVE", "1") == "1"
        # batch 0 setup + loop
        kfT0, qfT0 = setup_batch(0)

        sc_ps0 = ps_sc.tile([128, NK], F32, tag="sc", name="sc0")
        feature_loop(0, kfT0, qfT0, sc_ps0, build_onehot=True)
        if interleave:
            # batch 1 setup emitted before batch 0's output chain so the
            # engine streams don't block behind the exp dependency
            kfT1, qfT1 = setup_batch(1)
            finish_batch(0, sc_ps0)
        else:
            finish_batch(0, sc_ps0)
            kfT1, qfT1 = setup_batch(1)
        sc_ps1 = ps_sc.tile([128, NK], F32, tag="sc", name="sc1")
        feature_loop(1, kfT1, qfT1, sc_ps1, build_onehot=False)
        finish_batch(1, sc_ps1)

    nc.compile()
    return nc


_NC_CACHE = None
LAST_RESULTS = None


def kernel(queries, keys, values, valid_lens, W_q, W_k, w_v):
    global _NC_CACHE, LAST_RESULTS
    if _NC_CACHE is None:
        _NC_CACHE = _build()
    nc = _NC_CACHE

    queries = np.ascontiguousarray(queries, dtype=np.float32)
    keys = np.ascontiguousarray(keys, dtype=np.float32)
    values = np.ascontiguousarray(values, dtype=np.float32)
    valid_lens = np.ascontiguousarray(valid_lens, dtype=np.int32)
    W_q = np.ascontiguousarray(W_q, dtype=np.float32)
    W_k = np.ascontiguousarray(W_k, dtype=np.float32)
    w_v = np.ascontiguousarray(w_v, dtype=np.float32)

    in_maps = []
    for c in range(NCORES):
        lo, hi = c * BPC, (c + 1) * BPC
        in_maps.append(
            {
                "queries": queries[lo:hi],
                "keys": keys[lo:hi],
                "values": values[lo:hi],
                "valid_lens": valid_lens[lo:hi].reshape(BPC, 1),
                "W_q": W_q,
                "W_k": W_k,
                "w_v": w_v,
            }
        )

    trace = os.environ.get("ATTN_TRACE", "0") == "1"
    res = run_bass_kernel_spmd(
        nc, in_maps, core_ids=list(range(NCORES)), trace=trace
    )
    LAST_RESULTS = res
    return np.concatenate([r["out"] for r in res.results], axis=0)



# revision 2
# speedup vs baseline: 1.7444x; 1.7444x over previous
"""Additive attention (Bahdanau) TRN2 kernel, 8-core data parallel — v3.

score(q,k) = sum_h w_v[h] tanh(qf+kf) ~ sum_m B[m] sin(m W0 (qf+kf))
with m in {1,2,3,4,6,8,12,16}; each sinusoid separates into
sin*cos + cos*sin products -> PE matmul over (h, m, p).

Value generation (per element, C_m := 2 cos(m W0 x), W_m := 2 sin^2(m W0 x)):
  seeds s1, c1        : ACT Sin (args <= 1.3 rad)
  C1 = 2 c1           : DVE tensor_scalar (4x mode)
  W_m = Square(√2 s_m): ACT (any m we have s for)
  C_2m = 2 - 2 W_m    : DVE tensor_scalar affine
  s_2m = s_m * C_m    : DVE tensor_tensor (2x mode)
  s3, C3              : Chebyshev ladder on DVE
m=12,16 cos terms skip the affine entirely: C_2m = 2 - 2W_m and the
constant part is a per-row additive shift that softmax ignores, so the
matmul consumes W_m directly with coefficient -B[m] (host-folded).

Softmax tail at base partition 0 (batch on free dims); -30 mask bias and
the w_v*B/2 coefficient columns precomputed on host from valid_lens/w_v.
Normalization happens after attn @ V (linear), V and exp-scores feed
f32r matmuls directly.
"""

import os
from contextlib import ExitStack

import numpy as np

import concourse.bacc as bacc
import concourse.bass as bass
import concourse.mybir as mybir
import concourse.tile as tile
from concourse.bass_utils import run_bass_kernel_spmd

F32 = mybir.dt.float32
F32R = mybir.dt.float32r
BF16 = mybir.dt.bfloat16
AF = mybir.ActivationFunctionType
ALU = mybir.AluOpType

B, NQ, NK, QS, KS, H, VD = 16, 64, 512, 256, 256, 256, 256
NCORES = 8
BPC = B // NCORES
MASK_NEG = -30.0

MULTS = [1, 2, 3, 4, 6, 8, 12, 16]
NM = len(MULTS)
MIDX = {m: i for i, m in enumerate(MULTS)}
DOUBLINGS = [(2, 4), (3, 6), (4, 8), (6, 12), (8, 16)]
W0 = 0.22
COEF = [1.1742723433108437, 0.13354922600394753, 0.19147154488254783,
        0.17464585679464073, 0.116949727173799, 0.07173147367356891,
        0.03305145654189343, 0.006218691607399075]

SQRT2 = float(np.sqrt(2.0))
HALFPI = float(np.pi / 2)

APASS_POOL = os.environ.get("ATTN_APASS_POOL", "0") == "1"
DEBUG = os.environ.get("ATTN_DEBUG", "0") == "1"
# squares moved to DVE (as s*s products) for load balance: set of m
DVE_SQUARES = set(
    int(x) for x in os.environ.get("ATTN_DVE_SQ", "").split(",") if x
)
N_WARMUP = int(os.environ.get("ATTN_WARMUP", "12"))


def make_wvb(w_v):
    """A-side coefficient columns [128, 2ht, NM, 2{sin,cos}].

    sin column m pairs the key cos slab; cos column pairs the key sin slab.
    m=12/16: the cos slab is replaced by a PLAIN s^2 slab (squared on Pool),
    and C = 2 - 4 s^2, so the sin-A coefficient becomes -2b (the constant
    part is a per-row shift that softmax ignores).
    """
    wv2 = w_v.reshape(2, 128).T  # [p, ht]
    coef = np.asarray(COEF, dtype=np.float64)
    sin_col = wv2[:, :, None] * (coef[None, None, :] / 2)
    cos_col = wv2[:, :, None] * (coef[None, None, :] / 2)
    for m in (12, 16):
        sin_col[:, :, MIDX[m]] = wv2 * (-coef[MIDX[m]])
    return np.stack([sin_col, cos_col], axis=-1).astype(np.float32)


def _build():
    nc = bacc.Bacc()
    q_d = nc.declare_dram_parameter("queries", [BPC, NQ, QS], F32, isOutput=False)
    k_d = nc.declare_dram_parameter("keys", [BPC, NK, KS], F32, isOutput=False)
    v_d = nc.declare_dram_parameter("values", [BPC, NK, VD], F32, isOutput=False)
    wq_d = nc.declare_dram_parameter("W_q", [QS, H], F32, isOutput=False)
    wk_d = nc.declare_dram_parameter("W_k", [KS, H], F32, isOutput=False)
    wvb_d = nc.declare_dram_parameter("wvb", [128, 2, NM, 2], F32, isOutput=False)
    bias_d = nc.declare_dram_parameter("biasT", [1, BPC, NK], F32, isOutput=False)
    out_d = nc.declare_dram_parameter("out", [BPC, NQ, VD], F32, isOutput=True)

    ident_d = nc.inline_tensor(np.eye(128, dtype=np.float32), name="ident_c")
    dbg = {}
    if DEBUG:
        for nm, shape in [
            ("dbg_S1", [128, 2, 2, NK]), ("dbg_C1", [128, 2, 2, NK]),
            ("dbg_S16", [128, 2, 2, NK]), ("dbg_KW8", [128, 2, 2, NK]),
            ("dbg_A", [128, 2, NM, 2, 128]),
            ("dbg_Q1", [128, 2, 2, 128]), ("dbg_Q16", [128, 2, 2, 128]),
        ]:
            dbg[nm] = nc.declare_dram_parameter(nm, shape, BF16, isOutput=True)
        dbg["dbg_sc"] = nc.declare_dram_parameter("dbg_sc", [NQ, BPC, NK], F32,
                                                  isOutput=True)

    with ExitStack() as ctx:
        tc = ctx.enter_context(tile.TileContext(nc))
        consts = ctx.enter_context(tc.tile_pool(name="consts", bufs=1))
        setup = ctx.enter_context(tc.tile_pool(name="setup", bufs=1))
        chain = ctx.enter_context(tc.tile_pool(name="chain", bufs=1))
        sm = ctx.enter_context(tc.tile_pool(name="sm", bufs=1))
        ps_sc = ctx.enter_context(tc.tile_pool(name="ps_sc", bufs=2, space="PSUM"))

        # ---------------- DMA loads ----------------
        # sync: ident, keys b0, queries, wvb, bias ; gpsimd: keys b1, weights, V
        ident = consts.tile([128, 128], F32)
        nc.sync.dma_start(out=ident, in_=ident_d[:, :])
        keys_sb = []
        for b in range(BPC):
            k_sb = setup.tile([128, 4, KS], F32, name=f"k_sb{b}")
            keys_sb.append(k_sb)
        nc.sync.dma_start(
            out=keys_sb[0], in_=k_d[0].rearrange("(kb p) d -> p kb d", p=128)
        )
        nc.gpsimd.dma_start(
            out=keys_sb[1], in_=k_d[1].rearrange("(kb p) d -> p kb d", p=128)
        )
        q_sb = setup.tile([128, QS], F32, name="q_sb")
        nc.sync.dma_start(out=q_sb, in_=q_d.rearrange("b q d -> (b q) d"))
        wk_sb = consts.tile([128, 2, H], F32R, name="wk_sb")
        nc.gpsimd.dma_start(out=wk_sb, in_=wk_d.rearrange("(dt p) h -> p dt h", p=128))
        wq_sb = consts.tile([128, 2, H], F32R, name="wq_sb")
        nc.gpsimd.dma_start(out=wq_sb, in_=wq_d.rearrange("(dt p) h -> p dt h", p=128))
        wvb_sb = consts.tile([128, 2, NM, 2], F32, name="wvb_sb")
        nc.sync.dma_start(out=wvb_sb, in_=wvb_d[:, :, :, :])
        biasrow = sm.tile([1, BPC, NK], BF16, name="biasrow")
        nc.gpsimd.dma_start(out=biasrow, in_=bias_d[:, :, :])
        ones_bf = sm.tile([1, 64], BF16, name="ones_bf")
        nc.vector.memset(ones_bf, 1.0)
        v_sb = setup.tile([128, BPC, 4, VD], F32R, name="v_sb")
        halfpi = consts.tile([128, 1], F32)
        nc.vector.memset(halfpi, HALFPI)

        act, vec = nc.scalar, nc.vector

        # chain tiles
        # query: Q[m] [128, 2{s,C}, 2ht, 128(b q)] bf16 ; s half contiguous per ht? no:
        # layout [2p][2ht][128] so [:,p] is contiguous [2ht*128]=256 for chain ops,
        # and [:, :, ht] is the strided A-pass input.
        A = chain.tile([128, 2, NM, 2, 128], BF16, name="A")
        Q = {m: chain.tile([128, 2, 2, 128], BF16, name=f"Q{m}") for m in MULTS}
        QW = {m: chain.tile([128, 2, 128], BF16, name=f"QW{m}")
              for m in [1, 2, 3, 4, 6, 8]}
        qtmp1 = chain.tile([128, 2, 128], BF16, name="qtmp1")
        qtmp2 = chain.tile([128, 2, 128], BF16, name="qtmp2")
        qc1 = chain.tile([128, 2, 128], BF16, name="qc1")
        S = {m: chain.tile([128, 2, 2, NK], BF16, name=f"S{m}") for m in MULTS}
        CC = {m: chain.tile([128, 2, 2, NK], BF16, name=f"C{m}")
              for m in [1, 2, 3, 4, 6, 8]}  # no C12/C16 (W-trick)
        KW = {m: chain.tile([128, 2, 2, NK], BF16, name=f"KW{m}")
              for m in [1, 2, 3, 4, 6, 8]}
        ktmp1 = chain.tile([128, 2, 2, NK], BF16, name="ktmp1")
        ktmp2 = chain.tile([128, 2, 2, NK], BF16, name="ktmp2")
        kc1 = chain.tile([128, 2, 2, NK], BF16, name="kc1")

        apass_eng = nc.gpsimd if APASS_POOL else nc.vector

        def apass(m):
            mi = MIDX[m]
            if m in (12, 16):
                # sin/cos columns carry different coefficients (W-trick)
                for ht in range(2):
                    for p in range(2):
                        apass_eng.tensor_scalar_mul(
                            out=A[:, ht, mi, p],
                            in0=Q[m][:, p, ht],
                            scalar1=wvb_sb[:, ht, mi, p:p + 1],
                        )
            else:
                for ht in range(2):
                    apass_eng.tensor_scalar_mul(
                        out=A[:, ht, mi],
                        in0=Q[m][:, :, ht],
                        scalar1=wvb_sb[:, ht, mi, 0:1],
                    )

        # ---------------- PE warmup + queries (own psum scope) -----------
        with tc.tile_pool(name="ps_q", bufs=2, space="PSUM") as ps_q:
            warm = ps_q.tile([128, 128], F32, tag="psq", name="warm")
            for _ in range(N_WARMUP):
                nc.tensor.transpose(warm, ident, ident)
            ps_qT = ps_q.tile([128, 2, 128], F32, tag="psq2", name="ps_qT")
            for dt in range(2):
                nc.tensor.transpose(
                    ps_qT[:, dt], q_sb[:, dt * 128:(dt + 1) * 128], ident
                )
            qT_sb = setup.tile([128, 2, 128], F32R, name="qT_sb")
            nc.vector.tensor_copy(out=qT_sb, in_=ps_qT)
            ps_qf = ps_q.tile([128, 2, 128], F32, tag="psq2", name="ps_qf")
            for ht in range(2):
                for dt in range(2):
                    nc.tensor.matmul(
                        ps_qf[:, ht],
                        lhsT=wq_sb[:, dt, ht * 128:(ht + 1) * 128],
                        rhs=qT_sb[:, dt],
                        start=(dt == 0),
                        stop=(dt == 1),
                    )
            for ht in range(2):
                act.activation(out=Q[1][:, 0, ht], in_=ps_qf[:, ht],
                               func=AF.Sin, scale=W0)
            for ht in range(2):
                act.activation(out=qc1[:, ht], in_=ps_qf[:, ht], func=AF.Sin,
                               scale=W0, bias=halfpi[:, 0:1])
            act.activation(out=QW[1], in_=Q[1][:, 0], func=AF.Square, scale=SQRT2)

        # ---------------- keys + chains (scoped psum) ---------------------
        with tc.tile_pool(name="ps_kT", bufs=2, space="PSUM") as ps_kT, \
             tc.tile_pool(name="ps_kf", bufs=4, space="PSUM") as ps_kf:
            for b in range(BPC):
                nc.gpsimd.dma_start(
                    out=v_sb[:, b],
                    in_=v_d[b].rearrange("(kb p) d -> p kb d", p=128),
                )
            keysT = setup.tile([128, BPC, 2, NK], F32R, name="keysT")
            kfT_sb = setup.tile([128, BPC, 2, NK], F32, name="kfT_sb")
            for b in range(BPC):
                # transposes + evac for this batch
                psts = []
                for dt in range(2):
                    pst = ps_kT.tile([128, NK], F32, tag="pskT", name=f"pskT{b}{dt}")
                    for kb in range(4):
                        nc.tensor.transpose(
                            pst[:, kb * 128:(kb + 1) * 128],
                            keys_sb[b][:, kb, dt * 128:(dt + 1) * 128],
                            ident,
                        )
                    psts.append(pst)
                for dt in range(2):
                    nc.vector.tensor_copy(out=keysT[:, b, dt], in_=psts[dt])
                # projection per (b, ht); ACT copies psum -> kfT_sb so the
                # seeds can run as two big [128, 2048] Sin ops
                for ht in range(2):
                    pk = ps_kf.tile([128, NK], F32, tag="pskf", name=f"pskf{b}{ht}")
                    for dt in range(2):
                        nc.tensor.matmul(
                            pk,
                            lhsT=wk_sb[:, dt, ht * 128:(ht + 1) * 128],
                            rhs=keysT[:, b, dt],
                            start=(dt == 0),
                            stop=(dt == 1),
                        )
                    act.activation(out=kfT_sb[:, b, ht], in_=pk, func=AF.Copy)
            act.activation(out=S[1][:, :, :, :], in_=kfT_sb[:, :, :, :],
                           func=AF.Sin, scale=W0)
            act.activation(out=kc1[:, :, :, :], in_=kfT_sb[:, :, :, :],
                           func=AF.Sin, scale=W0, bias=halfpi[:, 0:1])

            # ---------------- score machinery ----------------
            sc_ps = [ps_sc.tile([NQ, NK], F32, tag="sc", name=f"sc{b}")
                     for b in range(BPC)]
            n_mm = [0] * BPC
            MM_TOTAL = NM * 2 * 2 + 1

            def emit_scores(m, p, slab):
                """p=0: sin slab (pairs cosA = A[...,1]); p=1: cos slab."""
                mi = MIDX[m]
                for b in range(BPC):
                    for ht in range(2):
                        nc.tensor.matmul(
                            sc_ps[b],
                            lhsT=A[:, ht, mi, 1 - p, b * 64:(b + 1) * 64],
                            rhs=slab[:, b, ht],
                            start=False,
                            stop=(n_mm[b] == MM_TOTAL - 1),
                        )
                        n_mm[b] += 1

            # ---------------- interleaved chains, dependency-ordered ------
            # invariant: apass(m) is emitted BEFORE the first emit_scores(m)
            def ksquare(m):
                act.activation(out=KW[m], in_=S[m], func=AF.Square, scale=SQRT2)

            def kaffine(src, dst):
                vec.tensor_scalar(out=CC[dst], in0=KW[src], scalar1=-2.0,
                                  scalar2=2.0, op0=ALU.mult, op1=ALU.add)

            def qsquare(m):
                act.activation(out=QW[m], in_=Q[m][:, 0], func=AF.Square,
                               scale=SQRT2)

            def qaffine(src, dst):
                vec.tensor_scalar(out=Q[dst][:, 1], in0=QW[src], scalar1=-2.0,
                                  scalar2=2.0, op0=ALU.mult, op1=ALU.add)

            def qproduct(src, dst):
                vec.tensor_tensor(out=Q[dst][:, 0], in0=Q[src][:, 0],
                                  in1=Q[src][:, 1], op=ALU.mult)

            vec.tensor_scalar_mul(out=CC[1], in0=kc1, scalar1=2.0)
            vec.tensor_scalar_mul(out=Q[1][:, 1], in0=qc1, scalar1=2.0)
            apass(1)
            # mask bias into the psum as a rank-1 matmul (ones x biasrow)
            for b in range(BPC):
                nc.tensor.matmul(
                    sc_ps[b],
                    lhsT=ones_bf[0:1, :],
                    rhs=biasrow[0:1, b],
                    start=True,
                    stop=False,
                )
                n_mm[b] += 1
            emit_scores(1, 0, S[1])
            emit_scores(1, 1, CC[1])
            ksquare(1)                                       # ACT W1
            vec.tensor_tensor(out=S[2], in0=S[1], in1=CC[1], op=ALU.mult)
            qaffine(1, 2)
            qproduct(1, 2)
            apass(2)
            emit_scores(2, 0, S[2])
            qsquare(2)                                       # ACT QW2
            kaffine(1, 2)
            emit_scores(2, 1, CC[2])
            ksquare(2)                                       # ACT W2
            # query ladder m3
            vec.tensor_tensor(out=qtmp1, in0=Q[1][:, 1], in1=Q[2][:, 0],
                              op=ALU.mult)
            vec.tensor_tensor(out=Q[3][:, 0], in0=qtmp1, in1=Q[1][:, 0],
                              op=ALU.subtract)
            vec.tensor_tensor(out=qtmp2, in0=Q[1][:, 1], in1=Q[2][:, 1],
                              op=ALU.mult)
            vec.tensor_tensor(out=Q[3][:, 1], in0=qtmp2, in1=Q[1][:, 1],
                              op=ALU.subtract)
            apass(3)
            # key ladder m3
            vec.tensor_tensor(out=ktmp1, in0=CC[1], in1=S[2], op=ALU.mult)
            vec.tensor_tensor(out=S[3], in0=ktmp1, in1=S[1], op=ALU.subtract)
            emit_scores(3, 0, S[3])
            qsquare(3)                                       # ACT QW3
            vec.tensor_tensor(out=ktmp2, in0=CC[1], in1=CC[2], op=ALU.mult)
            vec.tensor_tensor(out=CC[3], in0=ktmp2, in1=CC[1], op=ALU.subtract)
            emit_scores(3, 1, CC[3])
            ksquare(3)                                       # ACT W3
            # doublings: per dst, query side (and apass) first, then key side
            qk = {12: 6, 16: 8}
            for src, dst in DOUBLINGS:
                qaffine(src, dst)
                qproduct(src, dst)
                apass(dst)
                vec.tensor_tensor(out=S[dst], in0=S[src], in1=CC[src],
                                  op=ALU.mult)
                emit_scores(dst, 0, S[dst])
                if dst in (12, 16):
                    emit_scores(dst, 1, KW[qk[dst]])
                else:
                    qsquare(dst)                             # ACT QW[dst]
                    kaffine(src, dst)
                    emit_scores(dst, 1, CC[dst])
                    ksquare(dst)                         # ACT W[dst]

        if DEBUG:
            for nm, t in [("dbg_S1", S[1]), ("dbg_C1", CC[1]), ("dbg_S16", S[16]),
                          ("dbg_KW8", KW[8]), ("dbg_A", A), ("dbg_Q1", Q[1]),
                          ("dbg_Q16", Q[16])]:
                sl = tuple(slice(None) for _ in t.shape)
                nc.sync.dma_start(out=dbg[nm][sl], in_=t[sl])

        # ---------------- softmax + output ----------------
        e_sb = sm.tile([NQ, BPC, NK], F32, name="e_sb")
        den = sm.tile([NQ, BPC], F32, name="den")
        for b in range(BPC):
            nc.scalar.activation(out=e_sb[:, b], in_=sc_ps[b], func=AF.Exp,
                                 accum_out=den[:, b:b + 1])
        recip = sm.tile([NQ, BPC], F32, name="recip")
        nc.vector.reciprocal(recip, den)

        with tc.tile_pool(name="ps_tail", bufs=1, space="PSUM") as ps_tail:
            ps_aT = ps_tail.tile([128, BPC, 4, 64], F32, tag="tail", name="ps_aT")
            attnT = sm.tile([128, BPC, 4, 64], F32R, name="attnT")
            o_sb = sm.tile([NQ, BPC, VD], F32, name="o_sb")
            for b in range(BPC):
                for kb in range(4):
                    nc.tensor.transpose(
                        ps_aT[:, b, kb],
                        e_sb[:, b, kb * 128:(kb + 1) * 128],
                        ident[0:64, 0:64],
                    )
                nc.vector.tensor_copy(out=attnT[:, b], in_=ps_aT[:, b])
                po = ps_tail.tile([NQ, VD], F32, tag="tailo", bufs=2, name=f"po{b}")
                for kb in range(4):
                    nc.tensor.matmul(
                        po,
                        lhsT=attnT[:, b, kb],
                        rhs=v_sb[:, b, kb],
                        start=(kb == 0),
                        stop=(kb == 3),
                    )
                nc.vector.tensor_scalar_mul(
                    out=o_sb[:, b], in0=po, scalar1=recip[:, b:b + 1]
                )
                nc.sync.dma_start(out=out_d[b], in_=o_sb[:, b])

    nc.compile()
    return nc


_NC_CACHE = None
LAST_RESULTS = None


def kernel(queries, keys, values, valid_lens, W_q, W_k, w_v):
    global _NC_CACHE, LAST_RESULTS
    if _NC_CACHE is None:
        _NC_CACHE = _build()
    nc = _NC_CACHE

    queries = np.ascontiguousarray(queries, dtype=np.float32)
    keys = np.ascontiguousarray(keys, dtype=np.float32)
    values = np.ascontiguousarray(values, dtype=np.float32)
    valid_lens = np.ascontiguousarray(valid_lens, dtype=np.int32)
    W_q = np.ascontiguousarray(W_q, dtype=np.float32)
    W_k = np.ascontiguousarray(W_k, dtype=np.float32)
    w_v = np.ascontiguousarray(w_v, dtype=np.float32)

    wvb = make_wvb(w_v)
    karange = np.arange(NK)[None, :]

    in_maps = []
    for c in range(NCORES):
        lo, hi = c * BPC, (c + 1) * BPC
        vl = valid_lens[lo:hi]
        bias = np.where(karange < vl[:, None], 0.0, MASK_NEG).astype(np.float32)
        biasT = bias[None, :, :]
        in_maps.append(
            {
                "queries": queries[lo:hi],
                "keys": keys[lo:hi],
                "values": values[lo:hi],
                "W_q": W_q,
                "W_k": W_k,
                "wvb": np.ascontiguousarray(wvb),
                "biasT": np.ascontiguousarray(biasT),
            }
        )

    trace = os.environ.get("ATTN_TRACE", "0") == "1"
    res = run_bass_kernel_spmd(
        nc, in_maps, core_ids=list(range(NCORES)), trace=trace
    )
    LAST_RESULTS = res
    return np.concatenate([r["out"] for r in res.results], axis=0)
